# revision 1
# baseline (speedup 1.0000x reference)
"""Trainium2 Bass kernel: loss = 0.001 * ||diag(d^T d) - I||_F.

Contract: kernel(**inputs) takes the FULL input d [262144, 256] f32 and
returns the FULL scalar output, matching reference():

    col_sq = sum(d * d, axis=0)            # [256]
    loss   = 0.001 * sqrt(sum((col_sq - 1)^2))

Strategy (8 NeuronCores, row-sharded data parallel):
  - Shard d row-wise into 8 shards of [32768, 256], one per core.
  - Per core, stream [128, G*256] tiles from HBM and accumulate the
    per-column sum of squares.  Two compute paths:
      * "pe"  (default): gram-diagonal on the TensorEngine.  For each
        [128, 256] sub-tile S, matmul(S[:, 0:128].T @ S) and
        matmul(S[:, 128:256].T @ S) accumulate into two PSUM tiles whose
        diagonals are exactly the per-column sums of squares.  Squaring
        and the partition-dim reduction happen inside the PE MACs; the
        only non-PE work is the final PSUM->SBUF evacuation.  Uses
        float32r (full-rate fp32 path, 1 cycle/row for moving dim >=256).
      * "act": ScalarEngine Square + VectorEngine binary-tree folds into
        a [128, 256] accumulator, then a ones-vector fp32 matmul for the
        partition-dim reduction.  Exact fp32, used as numerics fallback.
  - Host: sum the 8 per-core partials in float64, extract diagonals
    (pe path), and finish the tiny scalar reduction.

Measured (8 cores streaming concurrently): ~105-116 us per core for the
full 32 MB pass (~300-330 GB/s/core sustained HBM; PE busy ~76 us and
hides under the DMA).  Rel err vs float64 reference: ~3e-9 — float32r's
reduced-mantissa products average out over the 262144-row reduction.
"""

import os
import sys

import numpy as np

for _p in ("/opt/trn_rl_repo",):
    if _p not in sys.path and os.path.isdir(_p):
        sys.path.insert(0, _p)

N_ROWS = 262144
M = 256
N_CORES = 8
SHARD = N_ROWS // N_CORES  # 32768 rows per core
P = 128  # SBUF partitions
G = 16  # [128, 256] sub-tiles per DMA'd big tile (2 MiB per DMA)

# Stash of the most recent BassKernelResults (test.py reads exec_time_ns).
LAST_RESULT = None

_programs = {}


def _build(path, bench_reps=1):
    import concourse.bacc as bacc
    import concourse.tile as tile
    from concourse import mybir

    f32 = mybir.dt.float32
    # float32r = fp32 storage on the TensorEngine's full-rate path (reduced
    # internal mantissa).  numpy-side dtype is float32 either way.
    d_dt = mybir.dt.float32r if path == "pe" else f32
    # Bacc (not raw Bass): its compile() legalizes multi-wait instructions
    # into event semaphores — TRN2 instructions carry at most one sem wait.
    nc = bacc.Bacc(trn_type="TRN2")
    d = nc.dram_tensor("d", [SHARD, M], d_dt, kind="ExternalInput").ap()
    n_big = SHARD // (P * G)
    assert n_big * P * G == SHARD
    # [t, p, g, m]: big-tile t, partition p, sub-tile g, column m.  Row-inner
    # mapping (G consecutive rows per partition) makes each partition's DMA
    # read 16 KiB contiguous — measured ~7 us/pass faster than 1 KiB chunks.
    # Any row->partition assignment is valid: the gram diagonals sum over all
    # rows regardless.
    dv = d.rearrange("(t p g) m -> t p g m", p=P, g=G)

    if path == "pe":
        out = nc.dram_tensor("out", [P, 2 * M], f32, kind="ExternalOutput").ap()
        with tile.TileContext(nc) as tc:
            with (
                tc.tile_pool(name="xs", bufs=4) as xs,
                tc.tile_pool(name="ps", bufs=1, space="PSUM") as ps,
                tc.tile_pool(name="outs", bufs=1) as outs,
            ):
                ps_a = ps.tile([P, M], f32)
                ps_b = ps.tile([P, M], f32)

                def full_pass():
                    for t in range(n_big):
                        xt = xs.tile([P, G, M], mybir.dt.float32r)
                        nc.sync.dma_start(out=xt, in_=dv[t])
                        for g in range(G):
                            sub = xt[:, g, :]
                            first = t == 0 and g == 0
                            last = t == n_big - 1 and g == G - 1
                            nc.tensor.matmul(
                                ps_a, sub[:, 0:P], sub, start=first, stop=last
                            )
                            nc.tensor.matmul(
                                ps_b, sub[:, P:M], sub, start=first, stop=last
                            )

                if bench_reps > 1:
                    # Benchmark mode: repeat the whole streaming pass in a HW
                    # loop; start=True re-clears PSUM so results stay valid.
                    with tc.For_i(0, bench_reps, 1):
                        full_pass()
                else:
                    full_pass()
                o = outs.tile([P, 2 * M], f32)
                nc.vector.tensor_copy(o[:, 0:M], ps_a)
                nc.vector.tensor_copy(o[:, M : 2 * M], ps_b)
                nc.sync.dma_start(out=out, in_=o)

        def post(outs_np):
            s = np.sum(np.asarray(outs_np, dtype=np.float64), axis=0)  # [128, 512]
            a, b = s[:, :M], s[:, M:]
            idx = np.arange(P)
            colsq = np.concatenate([a[idx, idx], b[idx, P + idx]])
            return colsq

    elif path == "act":
        out = nc.dram_tensor("out", [1, M], f32, kind="ExternalOutput").ap()
        with tile.TileContext(nc) as tc:
            with (
                tc.tile_pool(name="xs", bufs=3) as xs,
                tc.tile_pool(name="sq", bufs=2) as sqp,
                tc.tile_pool(name="acc", bufs=1) as accp,
                tc.tile_pool(name="ps", bufs=1, space="PSUM") as ps,
                tc.tile_pool(name="outs", bufs=1) as outs,
            ):
                acc = accp.tile([P, M], f32)
                ones = accp.tile([P, 1], f32)
                nc.vector.memset(acc, 0.0)
                nc.vector.memset(ones, 1.0)

                def full_pass():
                    for t in range(n_big):
                        xt = xs.tile([P, G * M], f32)
                        nc.sync.dma_start(
                            out=xt.rearrange("p (g m) -> p g m", g=G), in_=dv[t]
                        )
                        sq = sqp.tile([P, G * M], f32)
                        nc.scalar.activation(
                            sq, xt, mybir.ActivationFunctionType.Square
                        )
                        h = G * M // 2
                        while h >= M:
                            nc.vector.tensor_add(
                                sq[:, :h], sq[:, :h], sq[:, h : 2 * h]
                            )
                            h //= 2
                        nc.vector.tensor_add(acc, acc, sq[:, :M])

                if bench_reps > 1:
                    with tc.For_i(0, bench_reps, 1):
                        full_pass()
                else:
                    full_pass()
                # Partition-dim reduction: [1, 256] = ones[128,1].T @ acc.
                psum1 = ps.tile([1, M], f32)
                nc.tensor.matmul(psum1, ones, acc, start=True, stop=True)
                o = outs.tile([1, M], f32)
                nc.vector.tensor_copy(o, psum1)
                nc.sync.dma_start(out=out, in_=o)

        def post(outs_np):
            s = np.sum(np.asarray(outs_np, dtype=np.float64), axis=0)  # [1, 256]
            return s[0]

    else:
        raise ValueError(f"unknown path {path!r}")

    nc.compile()
    return nc, post


def _get_program(path):
    if path not in _programs:
        _programs[path] = _build(path)
    return _programs[path]


def kernel(d):
    global LAST_RESULT
    from concourse.bass_utils import run_bass_kernel_spmd

    d_np = np.ascontiguousarray(np.asarray(d, dtype=np.float32))
    assert d_np.shape == (N_ROWS, M), d_np.shape

    path = os.environ.get("BASS_KERNEL_PATH", "pe")
    nc, post = _get_program(path)

    shards = d_np.reshape(N_CORES, SHARD, M)
    in_maps = [{"d": np.ascontiguousarray(shards[i])} for i in range(N_CORES)]
    try:
        res = run_bass_kernel_spmd(nc, in_maps, core_ids=list(range(N_CORES)))
    except ModuleNotFoundError:
        # BASS_TRACE=1 under axon needs antenv.axon_hooks, which slim
        # containers lack — rerun untraced rather than crash.
        os.environ["BASS_NEVER_TRACE"] = "1"
        res = run_bass_kernel_spmd(nc, in_maps, core_ids=list(range(N_CORES)))
    LAST_RESULT = res

    colsq = post([r["out"] for r in res.results])
    loss = 0.001 * np.sqrt(np.sum((colsq - 1.0) ** 2))
    return np.asarray(loss, dtype=np.float32)



# revision 12
# speedup vs baseline: 1.0027x; 1.0027x over previous
"""Trainium2 Bass kernel: loss = 0.001 * ||diag(d^T d) - I||_F.

Contract: kernel(**inputs) takes the FULL input d [262144, 256] f32 and
returns the FULL scalar output, matching reference():

    col_sq = sum(d * d, axis=0)            # [256]
    loss   = 0.001 * sqrt(sum((col_sq - 1)^2))

Strategy (8 NeuronCores, row-sharded data parallel):
  - Shard d row-wise into 8 shards of [32768, 256], one per core.
  - Per core, stream [128, 16*256] 2 MiB tiles from HBM and accumulate the
    per-column sum of squares on the TensorEngine (gram-diagonal):
    for each [128, 256] sub-tile S, matmul(S[:, 0:128].T @ S) and
    matmul(S[:, 128:256].T @ S) accumulate into two PSUM tiles whose
    diagonals are exactly the per-column sums of squares.  float32r
    (full-rate fp32 path, 1 cycle/row since PSUM free dim = 256) keeps PE
    busy ~55-60 us — hidden under the DMA floor.
  - DMA: each tile is fetched as TWO 1 MiB halves, one from each HWDGE
    queue (SP/"sync" + Activation/"scalar"), 4-deep double buffering.
    A single queue sustains ~330 GB/s; the dual-queue half-split reaches
    ~360 GB/s/core = the 16-DMA-engine pool roofline (16 x 22.5 B/ns),
    i.e. ~93-98 us per 32 MiB pass vs ~102-116 us for the single-queue
    baseline.  (Tested worse: whole-tile round-robin across queues ~none,
    partition-dim split ~210 GB/s, gpsimd SWDGE 3rd queue ~88 GB/s slice
    drags the tile, g=8/32/64 tile sizes, bufs 3/5/6/8.)
  - Host: sum the 8 per-core partials in float64, extract diagonals,
    and finish the tiny scalar reduction.  Rel err vs float64 reference:
    ~1e-7 (float32r's reduced-mantissa products average out over the
    262144-row reduction).
"""

import os
import sys

import numpy as np

for _p in ("/opt/trn_rl_repo",):
    if _p not in sys.path and os.path.isdir(_p):
        sys.path.insert(0, _p)

N_ROWS = 262144
M = 256
N_CORES = 8
SHARD = N_ROWS // N_CORES  # 32768 rows per core
P = 128  # SBUF partitions

# Best measured config (see sweep.py): G sub-tiles per DMA'd big tile,
# BUFS-deep tile pool, DMA round-robined across SP+Act HWDGE queues.
BEST = dict(g=16, bufs=4, split="rr2")

# Stash of the most recent BassKernelResults (test.py reads exec_time_ns).
LAST_RESULT = None
# How many device executions the last kernel() call needed (sanity retries).
LAST_ATTEMPTS = 0

_programs = {}


def _build(path="pe", bench_reps=1, g=None, bufs=None, split=None):
    import concourse.bacc as bacc
    import concourse.tile as tile
    from concourse import mybir

    g = BEST["g"] if g is None else g
    bufs = BEST["bufs"] if bufs is None else bufs
    split = BEST["split"] if split is None else split

    f32 = mybir.dt.float32
    # float32r = fp32 storage on the TensorEngine's full-rate path (reduced
    # internal mantissa).  numpy-side dtype is float32 either way.
    d_dt = mybir.dt.float32r if path == "pe" else f32
    # Bacc (not raw Bass): its compile() legalizes multi-wait instructions
    # into event semaphores — TRN2 instructions carry at most one sem wait.
    nc = bacc.Bacc(trn_type="TRN2")
    d = nc.dram_tensor("d", [SHARD, M], d_dt, kind="ExternalInput").ap()
    n_big = SHARD // (P * g)
    assert n_big * P * g == SHARD
    # [t, p, g, m]: big-tile t, partition p, sub-tile g, column m.  Row-inner
    # mapping (g consecutive rows per partition) makes each partition's DMA
    # read g KiB contiguous.  Any row->partition assignment is valid: the
    # gram diagonals sum over all rows regardless.
    dv = d.rearrange("(t p g) m -> t p g m", p=P, g=g)

    def tile_dma(xt, t, dv):
        """Issue the DMAs bringing big-tile t into SBUF tile xt, spread
        across queues per `split`.  xt is [P, g, M], dv is [t, P, g, M]."""
        if split == "none":
            nc.sync.dma_start(out=xt, in_=dv[t])
        elif split == "rr2":
            [nc.sync, nc.scalar][t % 2].dma_start(out=xt, in_=dv[t])
        elif split.startswith("half2"):
            # "half2" → even split; "half2:9" → sync gets 9 of g sub-tiles.
            h = int(split.split(":")[1]) if ":" in split else g // 2
            nc.sync.dma_start(out=xt[:, 0:h, :], in_=dv[t, :, 0:h, :])
            nc.scalar.dma_start(out=xt[:, h:g, :], in_=dv[t, :, h:g, :])
        elif split == "half4":
            q = g // 4
            for i in range(4):
                eng = [nc.sync, nc.scalar][i % 2]
                eng.dma_start(
                    out=xt[:, i * q : (i + 1) * q, :],
                    in_=dv[t, :, i * q : (i + 1) * q, :],
                )
        elif split == "halfp":
            hp = P // 2
            nc.sync.dma_start(out=xt[0:hp], in_=dv[t, 0:hp])
            nc.scalar.dma_start(out=xt[hp:P], in_=dv[t, hp:P])
        elif split.startswith("half3"):
            # "half3" → even thirds; "half3:7:7" → sync 7, scalar 7, gpsimd
            # the remainder (SWDGE is ~4x slower, give it a small slice).
            if ":" in split:
                a, b = (int(x) for x in split.split(":")[1:])
                cuts = [0, a, a + b, g]
            else:
                cuts = [0, (g + 2) // 3, g - (g // 3), g]
            for i, eng in enumerate([nc.sync, nc.scalar, nc.gpsimd]):
                lo, hi = cuts[i], cuts[i + 1]
                eng.dma_start(out=xt[:, lo:hi, :], in_=dv[t, :, lo:hi, :])
        else:
            raise ValueError(split)

    if path == "pe":
        out = nc.dram_tensor("out", [P, 2 * M], f32, kind="ExternalOutput").ap()
        with tile.TileContext(nc) as tc:
            with (
                tc.tile_pool(name="xs", bufs=bufs) as xs,
                tc.tile_pool(name="ps", bufs=1, space="PSUM") as ps,
                tc.tile_pool(name="outs", bufs=1) as outs,
            ):
                ps_a = ps.tile([P, M], f32)
                ps_b = ps.tile([P, M], f32)

                def consume(xt, first_tile, last_tile):
                    for gi in range(g):
                        sub = xt[:, gi, :]
                        first = first_tile and gi == 0
                        last = last_tile and gi == g - 1
                        nc.tensor.matmul(
                            ps_a, sub[:, 0:P], sub, start=first, stop=last
                        )
                        nc.tensor.matmul(
                            ps_b, sub[:, P:M], sub, start=first, stop=last
                        )

                def full_pass():
                    if split == "dual":
                        # Each HWDGE queue independently streams half the
                        # shard (sync: tiles [0, n/2), scalar: [n/2, n)); PE
                        # alternates between the two streams.
                        nh = n_big // 2
                        for t in range(nh):
                            xa = xs.tile([P, g, M], mybir.dt.float32r)
                            nc.sync.dma_start(out=xa, in_=dv[t])
                            xb = xs.tile([P, g, M], mybir.dt.float32r)
                            nc.scalar.dma_start(out=xb, in_=dv[nh + t])
                            consume(xa, t == 0, False)
                            consume(xb, False, t == nh - 1)
                    else:
                        for t in range(n_big):
                            xt = xs.tile([P, g, M], mybir.dt.float32r)
                            tile_dma(xt, t, dv)
                            consume(xt, t == 0, t == n_big - 1)

                if bench_reps > 1:
                    # Benchmark mode: repeat the whole streaming pass in a HW
                    # loop; start=True re-clears PSUM so results stay valid.
                    with tc.For_i(0, bench_reps, 1):
                        full_pass()
                else:
                    full_pass()
                o = outs.tile([P, 2 * M], f32)
                nc.vector.tensor_copy(o[:, 0:M], ps_a)
                nc.vector.tensor_copy(o[:, M : 2 * M], ps_b)
                nc.sync.dma_start(out=out, in_=o)

        def post(outs_np):
            s = np.sum(np.asarray(outs_np, dtype=np.float64), axis=0)  # [128, 512]
            a, b = s[:, :M], s[:, M:]
            idx = np.arange(P)
            colsq = np.concatenate([a[idx, idx], b[idx, P + idx]])
            return colsq

    elif path == "act":
        out = nc.dram_tensor("out", [1, M], f32, kind="ExternalOutput").ap()
        with tile.TileContext(nc) as tc:
            with (
                tc.tile_pool(name="xs", bufs=3) as xs,
                tc.tile_pool(name="sq", bufs=2) as sqp,
                tc.tile_pool(name="acc", bufs=1) as accp,
                tc.tile_pool(name="ps", bufs=1, space="PSUM") as ps,
                tc.tile_pool(name="outs", bufs=1) as outs,
            ):
                acc = accp.tile([P, M], f32)
                ones = accp.tile([P, 1], f32)
                nc.vector.memset(acc, 0.0)
                nc.vector.memset(ones, 1.0)

                def full_pass():
                    for t in range(n_big):
                        xt = xs.tile([P, g * M], f32)
                        nc.sync.dma_start(
                            out=xt.rearrange("p (g m) -> p g m", g=g), in_=dv[t]
                        )
                        sq = sqp.tile([P, g * M], f32)
                        nc.scalar.activation(
                            sq, xt, mybir.ActivationFunctionType.Square
                        )
                        h = g * M // 2
                        while h >= M:
                            nc.vector.tensor_add(
                                sq[:, :h], sq[:, :h], sq[:, h : 2 * h]
                            )
                            h //= 2
                        nc.vector.tensor_add(acc, acc, sq[:, :M])

                if bench_reps > 1:
                    with tc.For_i(0, bench_reps, 1):
                        full_pass()
                else:
                    full_pass()
                # Partition-dim reduction: [1, 256] = ones[128,1].T @ acc.
                psum1 = ps.tile([1, M], f32)
                nc.tensor.matmul(psum1, ones, acc, start=True, stop=True)
                o = outs.tile([1, M], f32)
                nc.vector.tensor_copy(o, psum1)
                nc.sync.dma_start(out=out, in_=o)

        def post(outs_np):
            s = np.sum(np.asarray(outs_np, dtype=np.float64), axis=0)  # [1, 256]
            return s[0]

    else:
        raise ValueError(f"unknown path {path!r}")

    nc.compile()
    return nc, post


def _get_program(path):
    if path not in _programs:
        _programs[path] = _build(path)
    return _programs[path]


def kernel(d):
    global LAST_RESULT, LAST_ATTEMPTS
    from concourse.bass_utils import run_bass_kernel_spmd

    d_np = np.ascontiguousarray(np.asarray(d, dtype=np.float32))
    assert d_np.shape == (N_ROWS, M), d_np.shape

    path = os.environ.get("BASS_KERNEL_PATH", "pe")
    nc, post = _get_program(path)

    shards = d_np.reshape(N_CORES, SHARD, M)
    in_maps = [{"d": np.ascontiguousarray(shards[i])} for i in range(N_CORES)]

    def run_once():
        try:
            return run_bass_kernel_spmd(nc, in_maps, core_ids=list(range(N_CORES)))
        except ModuleNotFoundError:
            # BASS_TRACE=1 under axon needs antenv.axon_hooks, which slim
            # containers lack — rerun untraced rather than crash.
            os.environ["BASS_NEVER_TRACE"] = "1"
            return run_bass_kernel_spmd(nc, in_maps, core_ids=list(range(N_CORES)))

    # Sanity gate + retry: the very first device execution after another
    # process's heavy use has (rarely) returned garbage.  colsq is a vector
    # of sums of squares, so it must be finite, >= 0, and bounded by
    # n_rows * max(d^2); re-run the NEFF if the result is impossible.
    # Host-side only — does not touch the device in the nominal case.
    bound = 1.1 * N_ROWS * float(np.max(np.abs(d_np))) ** 2 + 1.0
    for _attempt in range(3):
        res = run_once()
        LAST_RESULT = res
        LAST_ATTEMPTS = _attempt + 1
        colsq = post([r["out"] for r in res.results])
        if (
            np.all(np.isfinite(colsq))
            and float(colsq.min()) >= -1.0
            and float(colsq.max()) <= bound
        ):
            break

    loss = 0.001 * np.sqrt(np.sum((colsq - 1.0) ** 2))
    return np.asarray(loss, dtype=np.float32)


# revision 18
# speedup vs baseline: 1.0368x; 1.0340x over previous
"""Trainium2 Bass kernel: loss = 0.001 * ||diag(d^T d) - I||_F.

Contract: kernel(**inputs) takes the FULL input d [262144, 256] f32 and
returns the FULL scalar output, matching reference():

    col_sq = sum(d * d, axis=0)            # [256]
    loss   = 0.001 * sqrt(sum((col_sq - 1)^2))

Strategy (8 NeuronCores, row-sharded data parallel):
  - Shard d row-wise into 8 shards of [32768, 256], one per core.
  - Per core, stream [128, 16*256] 2 MiB tiles from HBM and accumulate the
    per-column sum of squares on the TensorEngine (gram-diagonal):
    for each [128, 256] sub-tile S, matmul(S[:, 0:128].T @ S) and
    matmul(S[:, 128:256].T @ S) accumulate into two PSUM tiles whose
    diagonals are exactly the per-column sums of squares.  float32r
    (full-rate fp32 path, 1 cycle/row since PSUM free dim = 256) keeps PE
    busy ~55-60 us — hidden under the DMA floor.
  - DMA: each tile is fetched as FOUR contiguous 512 KiB HBM chunks,
    alternating between the two HWDGE queues (SP/"sync" + Activation/
    "scalar"), 4-deep double buffering.  A single queue sustains
    ~330 GB/s; dual-queue strided halves ~100 us; contiguous chunks with
    two outstanding DMAs per queue reach ~92-96 us per 32 MiB pass =
    ~360 GB/s/core, the 16-DMA-engine pool roofline (16 x 22.5 B/ns).
    (Tested worse: whole-tile round-robin across queues, partition-dim
    split ~210 GB/s, gpsimd SWDGE 3rd queue ~88 GB/s slice drags the
    tile, 8-chunk split ~tied, g=8/32/64 tile sizes, bufs 3/5/6/8.)
  - Host: sum the 8 per-core partials in float64, extract diagonals,
    and finish the tiny scalar reduction.  Rel err vs float64 reference:
    ~1e-7 (float32r's reduced-mantissa products average out over the
    262144-row reduction).
"""

import os
import sys

import numpy as np

for _p in ("/opt/trn_rl_repo",):
    if _p not in sys.path and os.path.isdir(_p):
        sys.path.insert(0, _p)

N_ROWS = 262144
M = 256
N_CORES = 8
SHARD = N_ROWS // N_CORES  # 32768 rows per core
P = 128  # SBUF partitions

# Best measured config (see sweep.py): 2 MiB big tiles (g=16 sub-tiles),
# 4-deep tile pool, each tile fetched as 4 contiguous 512 KiB chunks
# alternating between the SP and Activation HWDGE queues.
BEST = dict(g=16, bufs=4, split="halfc:4")

# Stash of the most recent BassKernelResults (test.py reads exec_time_ns).
LAST_RESULT = None
# How many device executions the last kernel() call needed (sanity retries).
LAST_ATTEMPTS = 0

_programs = {}


def _build(path="pe", bench_reps=1, g=None, bufs=None, split=None):
    import concourse.bacc as bacc
    import concourse.tile as tile
    from concourse import mybir

    g = BEST["g"] if g is None else g
    bufs = BEST["bufs"] if bufs is None else bufs
    split = BEST["split"] if split is None else split

    f32 = mybir.dt.float32
    # float32r = fp32 storage on the TensorEngine's full-rate path (reduced
    # internal mantissa).  numpy-side dtype is float32 either way.
    d_dt = mybir.dt.float32r if path == "pe" else f32
    # Bacc (not raw Bass): its compile() legalizes multi-wait instructions
    # into event semaphores — TRN2 instructions carry at most one sem wait.
    nc = bacc.Bacc(trn_type="TRN2")
    d = nc.dram_tensor("d", [SHARD, M], d_dt, kind="ExternalInput").ap()
    n_big = SHARD // (P * g)
    assert n_big * P * g == SHARD
    # [t, p, g, m]: big-tile t, partition p, sub-tile g, column m.  Row-inner
    # mapping (g consecutive rows per partition) makes each partition's DMA
    # read g KiB contiguous.  Any row->partition assignment is valid: the
    # gram diagonals sum over all rows regardless.
    dv = d.rearrange("(t p g) m -> t p g m", p=P, g=g)

    # Chunked views: [t, f, p, q, m] — chunk f of big-tile t is a fully
    # contiguous HBM range (vs the used/skipped stride of dv's sub-slices).
    # Row order differs from dv; the column sums of squares don't care.
    _views = {}

    def chunked_view(n):
        if n not in _views:
            _views[n] = d.rearrange(
                "(t f p q) m -> t f p q m", f=n, p=P, q=g // n
            )
        return _views[n]

    def tile_dma(xt, t, dv):
        """Issue the DMAs bringing big-tile t into SBUF tile xt, spread
        across queues per `split`.  xt is [P, g, M], dv is [t, P, g, M]."""
        if split == "none":
            nc.sync.dma_start(out=xt, in_=dv[t])
        elif split == "rr2":
            [nc.sync, nc.scalar][t % 2].dma_start(out=xt, in_=dv[t])
        elif split.startswith("halfc:"):
            # n contiguous chunks per tile, queues alternating: each queue
            # issues n/2 DMAs per tile, every DMA a contiguous HBM range.
            n = int(split.split(":")[1])
            dvn = chunked_view(n)
            q = g // n
            for i in range(n):
                eng = [nc.sync, nc.scalar][i % 2]
                eng.dma_start(out=xt[:, i * q : (i + 1) * q, :], in_=dvn[t, i])
        elif split.startswith("half2"):
            # "half2" → even split; "half2:9" → sync gets 9 of g sub-tiles.
            h = int(split.split(":")[1]) if ":" in split else g // 2
            nc.sync.dma_start(out=xt[:, 0:h, :], in_=dv[t, :, 0:h, :])
            nc.scalar.dma_start(out=xt[:, h:g, :], in_=dv[t, :, h:g, :])
        elif split == "half4":
            q = g // 4
            for i in range(4):
                eng = [nc.sync, nc.scalar][i % 2]
                eng.dma_start(
                    out=xt[:, i * q : (i + 1) * q, :],
                    in_=dv[t, :, i * q : (i + 1) * q, :],
                )
        elif split == "halfp":
            hp = P // 2
            nc.sync.dma_start(out=xt[0:hp], in_=dv[t, 0:hp])
            nc.scalar.dma_start(out=xt[hp:P], in_=dv[t, hp:P])
        elif split.startswith("half3"):
            # "half3" → even thirds; "half3:7:7" → sync 7, scalar 7, gpsimd
            # the remainder (SWDGE is ~4x slower, give it a small slice).
            if ":" in split:
                a, b = (int(x) for x in split.split(":")[1:])
                cuts = [0, a, a + b, g]
            else:
                cuts = [0, (g + 2) // 3, g - (g // 3), g]
            for i, eng in enumerate([nc.sync, nc.scalar, nc.gpsimd]):
                lo, hi = cuts[i], cuts[i + 1]
                eng.dma_start(out=xt[:, lo:hi, :], in_=dv[t, :, lo:hi, :])
        else:
            raise ValueError(split)

    if path == "pe":
        out = nc.dram_tensor("out", [P, 2 * M], f32, kind="ExternalOutput").ap()
        with tile.TileContext(nc) as tc:
            with (
                tc.tile_pool(name="xs", bufs=bufs) as xs,
                tc.tile_pool(name="ps", bufs=1, space="PSUM") as ps,
                tc.tile_pool(name="outs", bufs=1) as outs,
            ):
                ps_a = ps.tile([P, M], f32)
                ps_b = ps.tile([P, M], f32)

                def consume(xt, first_tile, last_tile):
                    for gi in range(g):
                        sub = xt[:, gi, :]
                        first = first_tile and gi == 0
                        last = last_tile and gi == g - 1
                        nc.tensor.matmul(
                            ps_a, sub[:, 0:P], sub, start=first, stop=last
                        )
                        nc.tensor.matmul(
                            ps_b, sub[:, P:M], sub, start=first, stop=last
                        )

                def full_pass():
                    if split == "dual":
                        # Each HWDGE queue independently streams half the
                        # shard (sync: tiles [0, n/2), scalar: [n/2, n)); PE
                        # alternates between the two streams.
                        nh = n_big // 2
                        for t in range(nh):
                            xa = xs.tile([P, g, M], mybir.dt.float32r)
                            nc.sync.dma_start(out=xa, in_=dv[t])
                            xb = xs.tile([P, g, M], mybir.dt.float32r)
                            nc.scalar.dma_start(out=xb, in_=dv[nh + t])
                            consume(xa, t == 0, False)
                            consume(xb, False, t == nh - 1)
                    else:
                        for t in range(n_big):
                            xt = xs.tile([P, g, M], mybir.dt.float32r)
                            tile_dma(xt, t, dv)
                            consume(xt, t == 0, t == n_big - 1)

                if bench_reps > 1:
                    # Benchmark mode: repeat the whole streaming pass in a HW
                    # loop; start=True re-clears PSUM so results stay valid.
                    with tc.For_i(0, bench_reps, 1):
                        full_pass()
                else:
                    full_pass()
                o = outs.tile([P, 2 * M], f32)
                nc.vector.tensor_copy(o[:, 0:M], ps_a)
                nc.vector.tensor_copy(o[:, M : 2 * M], ps_b)
                nc.sync.dma_start(out=out, in_=o)

        def post(outs_np):
            s = np.sum(np.asarray(outs_np, dtype=np.float64), axis=0)  # [128, 512]
            a, b = s[:, :M], s[:, M:]
            idx = np.arange(P)
            colsq = np.concatenate([a[idx, idx], b[idx, P + idx]])
            return colsq

    elif path == "act":
        out = nc.dram_tensor("out", [1, M], f32, kind="ExternalOutput").ap()
        with tile.TileContext(nc) as tc:
            with (
                tc.tile_pool(name="xs", bufs=3) as xs,
                tc.tile_pool(name="sq", bufs=2) as sqp,
                tc.tile_pool(name="acc", bufs=1) as accp,
                tc.tile_pool(name="ps", bufs=1, space="PSUM") as ps,
                tc.tile_pool(name="outs", bufs=1) as outs,
            ):
                acc = accp.tile([P, M], f32)
                ones = accp.tile([P, 1], f32)
                nc.vector.memset(acc, 0.0)
                nc.vector.memset(ones, 1.0)

                def full_pass():
                    for t in range(n_big):
                        xt = xs.tile([P, g * M], f32)
                        nc.sync.dma_start(
                            out=xt.rearrange("p (g m) -> p g m", g=g), in_=dv[t]
                        )
                        sq = sqp.tile([P, g * M], f32)
                        nc.scalar.activation(
                            sq, xt, mybir.ActivationFunctionType.Square
                        )
                        h = g * M // 2
                        while h >= M:
                            nc.vector.tensor_add(
                                sq[:, :h], sq[:, :h], sq[:, h : 2 * h]
                            )
                            h //= 2
                        nc.vector.tensor_add(acc, acc, sq[:, :M])

                if bench_reps > 1:
                    with tc.For_i(0, bench_reps, 1):
                        full_pass()
                else:
                    full_pass()
                # Partition-dim reduction: [1, 256] = ones[128,1].T @ acc.
                psum1 = ps.tile([1, M], f32)
                nc.tensor.matmul(psum1, ones, acc, start=True, stop=True)
                o = outs.tile([1, M], f32)
                nc.vector.tensor_copy(o, psum1)
                nc.sync.dma_start(out=out, in_=o)

        def post(outs_np):
            s = np.sum(np.asarray(outs_np, dtype=np.float64), axis=0)  # [1, 256]
            return s[0]

    else:
        raise ValueError(f"unknown path {path!r}")

    nc.compile()
    return nc, post


def _get_program(path):
    if path not in _programs:
        _programs[path] = _build(path)
    return _programs[path]


def kernel(d):
    global LAST_RESULT, LAST_ATTEMPTS
    from concourse.bass_utils import run_bass_kernel_spmd

    d_np = np.ascontiguousarray(np.asarray(d, dtype=np.float32))
    assert d_np.shape == (N_ROWS, M), d_np.shape

    path = os.environ.get("BASS_KERNEL_PATH", "pe")
    nc, post = _get_program(path)

    shards = d_np.reshape(N_CORES, SHARD, M)
    in_maps = [{"d": np.ascontiguousarray(shards[i])} for i in range(N_CORES)]

    def run_once():
        try:
            return run_bass_kernel_spmd(nc, in_maps, core_ids=list(range(N_CORES)))
        except ModuleNotFoundError:
            # BASS_TRACE=1 under axon needs antenv.axon_hooks, which slim
            # containers lack — rerun untraced rather than crash.
            os.environ["BASS_NEVER_TRACE"] = "1"
            return run_bass_kernel_spmd(nc, in_maps, core_ids=list(range(N_CORES)))

    # Sanity gate + retry: the very first device execution after another
    # process's heavy use has (rarely) returned garbage.  colsq is a vector
    # of sums of squares, so it must be finite, >= 0, and bounded by
    # n_rows * max(d^2); re-run the NEFF if the result is impossible.
    # Host-side only — does not touch the device in the nominal case.
    bound = 1.1 * N_ROWS * float(np.max(np.abs(d_np))) ** 2 + 1.0
    for _attempt in range(3):
        res = run_once()
        LAST_RESULT = res
        LAST_ATTEMPTS = _attempt + 1
        colsq = post([r["out"] for r in res.results])
        if (
            np.all(np.isfinite(colsq))
            and float(colsq.min()) >= -1.0
            and float(colsq.max()) <= bound
        ):
            break

    loss = 0.001 * np.sqrt(np.sum((colsq - 1.0) ** 2))
    return np.asarray(loss, dtype=np.float32)
